# revision 45
# baseline (speedup 1.0000x reference)
"""Trainium2 Bass kernel for axial (per-frame) spatial multi-head attention.

Computation (per batch element b):
    qkv = x @ Wqkv ; q,k,v heads of 64 dims, q scaled by D**-0.5
    per (head, frame): attn = softmax(q @ k^T) over 196 spatial tokens
    out = attn @ v ; y = concat-heads(out) @ Wout + bout

Sharding: pure data-parallel over batch B=8 -> one NeuronCore per batch
element, no collectives. Each core computes its full [1568, 512] output.

Single-core dataflow (no on-device transposes anywhere):
  - host supplies x^T [512,1568] fp16; q/k produced TRANSPOSED (qT/kT
    [64h, t]) with Wq/k slices stationary; v produced NATURAL with xT
    slices stationary. All PE matmuls fp16/bf16 (1 cy/row).
  - attention is FRAME-MAJOR (unit u = 4*frame + pair) and software-
    pipelined: sim(u) at step u, AV(u) at lag 2, normalize at lag 3.
  - sim per (pair, frame): four K=64 matmuls alternating PE quadrants
    (head-even rows 0:64, head-odd 64:128 -> loads hide under streaming);
    one ACT exp (bias=-SHIFT) over both heads writes attnwT bf16.
  - AV contracts j on partitions with a per-head ones column in v; row 64
    of the psum output is the softmax denominator.
  - normalize: the denominator row is bounced to a base-0 SBUF row
    (ACT/DVE alternating; the custom-DVE reciprocal faults on PSUM
    operands and silently corrupts on base!=0 operands), DVE
    reciprocal_approx_fast, GpSimd partition_broadcast (GpSimd cannot
    touch PSUM and must run only this one op kind - mixing Pool op types
    costs a ~6us engine reconfig per switch), then two lane-shifted DVE
    muls write the normalized out^T fp16 tile.
  - scheduling (the TRN2 clock governor runs k=8/full speed only after
    several us of near-continuous PE busy; any gap over ~1.5us drops it
    to k=4/half speed - visible as the ham records in the NTFF profile):
    the projection and out-proj chains are spread UNIFORMLY across the
    whole attention pipeline instead of front-loaded - one q/k chain per
    step through step 25 (each chunk's kT chains 4 steps before its qT
    chains: the jc1 stationary window of frame 2*nch-1 reads 60 columns
    into chunk nch), v chains on even steps, one out-proj chain per step
    at its earliest data-ready step. Frames run in order 0-5,7,6 (see
    FORDER): frame 6 goes last so only TWO chains (mt9, mt10) gate on
    the final norms - they emit their matmuls INCREMENTALLY (matmul p
    right after norm(28+p), steps 31-34) while the frame-7-only chains
    mt11/mt12 retire early inside the former tail-gap region.
    sims 0-3 start inside the opening region right after their q/k
    chains drain; av units 30-31 borrow the idle sim psum tiles so they
    skip the ~2.5us norm-chain wait for a pav slot.
  - startup: the opening kc-outer block needs only wqk[kc][:, 0:512]
    (pairs 0,1) + xt[kc] nch0, so the first DMAs are a 128KB wqk0 half
    and the 100KB x chunk; wqk is striped over both queues. ZERO-
    STATIONARY filler matmuls (accumulate +0 into the open psum groups)
    pad the opener's DMA-arrival stalls so the clock ramps by ~14us
    instead of ~27us - worth ~4us of total runtime.
  - DMA: weights on the ACT queue, x^T + stores on SP. GpSimd-issued
    DMA silently corrupts on this runtime; DVE/PE cannot issue DMA.
  - measured dead ends (all slower on hw): 8-wide kc-outer opener across
    all 8 psum banks (+18us!); nc.vector.reciprocal instead of the
    custom approx_fast (+60us); pair-merged recip/broadcast (+14us,
    bursty DVE); deferring v chains into the tail; norm-before-av
    emission; zero-fills inside the tail out-proj groups. The engine-
    balance here (DVE ~58%, ACT ~56%, PE ~92% of the window) is a sharp
    local optimum - single-step schedule shuffles swing +-5-15us.
"""

import numpy as np

B, N, DIM = 8, 1568, 512
H, D, F = 8, 64, 8
NTOK = 196          # spatial tokens per frame
TCH = 392           # token chunk (2 frames), 4*392=1568
KC = 4              # 128-row chunks over DIM contraction
SHIFT = 90.0        # softmax exp shift (see module docstring)
VSTR = 65           # per-head stride in v_aug (64 dims + ones column)
NU = 4 * F          # attention units (pair, frame-slot)
FORDER = (0, 1, 2, 3, 4, 5, 7, 6)  # frame processed at each slot: frame 6
# goes LAST so only two out-proj chains (mt9, mt10) gate on the final
# norms; the frame-7-only chains (mt11, mt12) retire early and their
# matmuls/drains/stores fill what used to be the tail's idle gaps

_cache = {}


def _build_bass(use_bias: bool):
    import concourse.tile as tile
    import concourse.mybir as mybir
    from concourse import bacc

    fp32 = mybir.dt.float32
    fp16 = mybir.dt.float16
    bf16 = mybir.dt.bfloat16
    Exp = mybir.ActivationFunctionType.Exp

    nc = bacc.Bacc()
    xT_d = nc.declare_dram_parameter("xT", [DIM, N], fp16, isOutput=False)
    wqkv_d = nc.declare_dram_parameter("wqkv", [DIM, 3 * DIM], fp16, isOutput=False)
    wout_d = nc.declare_dram_parameter("wout", [DIM, DIM], fp16, isOutput=False)
    if use_bias:
        bout_d = nc.declare_dram_parameter("boutr", [1, DIM], fp16, isOutput=False)
    out_d = nc.declare_dram_parameter("out", [N, DIM], fp16, isOutput=True)

    with tile.TileContext(nc) as tc:
        with (
            tc.tile_pool(name="weights", bufs=1) as wpool,
            tc.tile_pool(name="acts", bufs=1) as apool,
            tc.tile_pool(name="attnw", bufs=8) as atpool,
            tc.tile_pool(name="rows", bufs=4) as rpool,
            tc.tile_pool(name="ys", bufs=6) as yspool,
            tc.tile_pool(name="pmm", bufs=2, space="PSUM") as pmm,
            tc.tile_pool(name="psim", bufs=2, space="PSUM") as psim,
            tc.tile_pool(name="pav", bufs=2, space="PSUM") as pav,
        ):
            # ---- resident tiles ----
            # host packs wqkv columns pair-major: [q_p0|k_p0|q_p1|k_p1|...|v]
            wqk = [wpool.tile([128, 2 * DIM], fp16, tag=f"wqk_{kc}",
                              name=f"wqk_{kc}") for kc in range(KC)]
            xt = [wpool.tile([128, N], fp16, tag=f"xt_{kc}", name=f"xt_{kc}")
                  for kc in range(KC)]
            wv = [wpool.tile([128, DIM], fp16, tag=f"wv_{kc}", name=f"wv_{kc}")
                  for kc in range(KC)]
            wout = [wpool.tile([128, DIM], fp16, tag=f"wout_{p}",
                               name=f"wout_{p}") for p in range(4)]

            # ---- DMA issue plan (see module docstring) ----
            # ACT queue: wqk0 halves first (opening block), then wqk2/3, wv,
            # wout. SP queue: xt nch0 chunks interleaved with wqk1 halves,
            # then xt nch1, then merged nch2+3.
            nc.scalar.dma_start(out=wqk[0][:, 0:512], in_=wqkv_d[0:128, 0:512])
            nc.sync.dma_start(out=xt[0][:, 0:TCH], in_=xT_d[0:128, 0:TCH])
            nc.scalar.dma_start(out=wqk[0][:, 512:1024],
                                in_=wqkv_d[0:128, 512:1024])
            nc.sync.dma_start(out=wqk[1][:, 0:512], in_=wqkv_d[128:256, 0:512])
            nc.scalar.dma_start(out=wqk[2][:, 0:512],
                                in_=wqkv_d[256:384, 0:512])
            nc.sync.dma_start(out=xt[1][:, 0:TCH], in_=xT_d[128:256, 0:TCH])
            nc.scalar.dma_start(out=wqk[2][:, 512:1024],
                                in_=wqkv_d[256:384, 512:1024])
            nc.sync.dma_start(out=wqk[1][:, 512:1024],
                              in_=wqkv_d[128:256, 512:1024])
            nc.sync.dma_start(out=xt[2][:, 0:TCH], in_=xT_d[256:384, 0:TCH])
            nc.sync.dma_start(out=xt[3][:, 0:TCH], in_=xT_d[384:512, 0:TCH])
            nc.sync.dma_start(out=wqk[3][:, 0:512], in_=wqkv_d[384:512, 0:512])
            nc.sync.dma_start(out=wqk[3][:, 512:1024],
                              in_=wqkv_d[384:512, 512:1024])
            for kc in range(KC):
                nc.scalar.dma_start(
                    out=wv[kc][:], in_=wqkv_d[kc * 128:(kc + 1) * 128,
                                              2 * DIM:3 * DIM]
                )
            for kc in range(KC):
                nc.sync.dma_start(
                    out=xt[kc][:, TCH:2 * TCH],
                    in_=xT_d[kc * 128:(kc + 1) * 128, TCH:2 * TCH],
                )
            for p in range(4):
                nc.scalar.dma_start(out=wout[p][:],
                                    in_=wout_d[p * 128:(p + 1) * 128, :])
            for kc in range(KC):
                nc.sync.dma_start(
                    out=xt[kc][:, 2 * TCH:N],
                    in_=xT_d[kc * 128:(kc + 1) * 128, 2 * TCH:N],
                )
            if use_bias:
                boutt = wpool.tile([1, DIM], fp16, tag="boutr", name="boutr")
                nc.scalar.dma_start(out=boutt[:], in_=bout_d[:])
                ones_r = wpool.tile([1, 128], fp16, tag="ones_r", name="ones_r")
                nc.gpsimd.memset(ones_r[:], 1.0)
            negshift = wpool.tile([128, 1], fp32, tag="negshift", name="negshift")
            nc.gpsimd.memset(negshift[:], -SHIFT)

            # qT tiles m=0..3 (pair m heads 2m,2m+1); kT tiles m=4..7 with 64
            # zero pad columns so the jc1 stationary slice of the last frame
            # stays in bounds (rows 68:128 of jc1 sim output are garbage,
            # never read downstream).
            qkvT = [apool.tile([128, N if m < 4 else N + 64], fp16,
                               tag=f"qkvT_{m}", name=f"qkvT_{m}")
                    for m in range(8)]
            for m in range(4, 8):
                nc.gpsimd.memset(qkvT[m][:, N:N + 64], 0.0)
            vaug = []
            for fr in range(F):
                pair = []
                for c, rows in ((0, 128), (1, 68)):
                    t = apool.tile([rows, H * VSTR], bf16, tag=f"vaug_{fr}_{c}",
                                   name=f"vaug_{fr}_{c}")
                    nc.gpsimd.memset(
                        t[:].rearrange("p (h c) -> p h c", h=H)[:, :, 64:65], 1.0
                    )
                    pair.append(t)
                vaug.append(pair)
            outT = [apool.tile([128, N], fp16, tag=f"outT_{p}", name=f"outT_{p}")
                    for p in range(4)]

            drains = [nc.scalar.copy, nc.vector.tensor_copy]
            _di = [0]

            def drain():
                _di[0] += 1
                return drains[_di[0] % 2]

            def qk_chain(m, nch, nfill=0):
                col0 = 256 * m if m < 4 else 256 * (m - 4) + 128
                ps = pmm.tile([128, DIM], fp32, tag="mm", name="chps")
                if nfill:
                    fill(ps[:, 0:TCH], nfill, TCH, start=True)
                for kc in range(KC):
                    nc.tensor.matmul(
                        ps[:, 0:TCH],
                        wqk[kc][:, col0:col0 + 128],
                        xt[kc][:, nch * TCH:(nch + 1) * TCH],
                        start=(kc == 0 and not nfill), stop=(kc == KC - 1),
                    )
                drain()(qkvT[m][:, nch * TCH:(nch + 1) * TCH], ps[:, 0:TCH])

            def v_chain(fr, c, nfill=0):
                rows = 128 if c == 0 else 68
                tok0 = fr * NTOK + c * 128
                ps = pmm.tile([128, DIM], fp32, tag="mm", name="chps")
                if nfill:
                    fill(ps[:, 0:DIM], nfill, DIM, start=True)
                for kc in range(KC):
                    nc.tensor.matmul(
                        ps[0:rows, :],
                        xt[kc][:, tok0:tok0 + rows],
                        wv[kc][:],
                        start=(kc == 0 and not nfill), stop=(kc == KC - 1),
                    )
                drain()(
                    vaug[fr][c][:].rearrange("p (h c) -> p h c", h=H)[:, :, 0:64],
                    ps[0:rows, :].rearrange("p (h c) -> p h c", h=H),
                )

            # ---- attention pipeline units ----
            at_t, avs_t, rr_t, dsb_t = {}, {}, {}, {}

            # zero-stationary filler matmuls: accumulate +0 into an open
            # PSUM group while a weight/x DMA is still in flight, so the PE
            # stream never breaks during startup (any multi-us idle drops
            # the clock state and the re-ramp costs ~5us at half speed)
            zstat = wpool.tile([128, 128], fp16, tag="zstat", name="zstat")
            zmov = wpool.tile([128, DIM], fp16, tag="zmov", name="zmov")
            nc.gpsimd.memset(zstat[:], 0.0)
            nc.gpsimd.memset(zmov[:], 0.0)

            def fill(ps_ap, n, w, start=False):
                for i in range(n):
                    nc.tensor.matmul(
                        ps_ap, zstat[:], zmov[:, 0:w],
                        start=(start and i == 0), stop=False,
                    )

            def sim_unit(u):
                g, p = divmod(u, 4)
                fr = FORDER[g]
                c0 = fr * NTOK
                # head blocks at 512-col stride: each head's 392 sim columns
                # stay inside one 2KB PSUM bank (matmul dst cannot span banks)
                ps = psim.tile([128, 1024], fp32, tag="sim", name="sim")
                qTt, kTt = qkvT[p], qkvT[4 + p]
                for hh, jc in ((0, 0), (1, 0), (0, 1), (1, 1)):
                    base = hh * 64
                    off = hh * 512 + jc * NTOK
                    nc.tensor.matmul(
                        ps[0:128, off:off + NTOK],
                        kTt[base:base + 64, c0 + jc * 128:c0 + jc * 128 + 128],
                        qTt[base:base + 64, c0:c0 + NTOK],
                    )
                at = atpool.tile([128, 2 * TCH], bf16, tag="at", name="at")
                nc.scalar.activation(
                    at[:].rearrange("p (b c) -> p b c", b=2),
                    ps[:].rearrange("p (b c) -> p b c", b=2)[:, :, 0:TCH],
                    Exp,
                    bias=negshift[:],
                )
                at_t[u] = at

            def av_unit(u):
                g, p = divmod(u, 4)
                fr = FORDER[g]
                at = at_t.pop(u)
                if u >= 30:
                    # sims have stopped allocating by now: borrow the sim
                    # psum tiles so the last av units don't wait on the
                    # ~2.5us norm chain to free a pav slot (the tail's PE
                    # gaps drop the clock to half speed)
                    av = psim.tile([128, 1024], fp32, tag="sim", name="av")
                else:
                    av = pav.tile([128, DIM], fp32, tag="av", name="av")
                for hh in range(2):
                    ato = hh * TCH
                    avo = hh * NTOK
                    for c, rows in ((0, 128), (1, 68)):
                        va = vaug[fr][c][:].rearrange(
                            "p (h c) -> p h c", h=H)[:, 2 * p + hh, :]
                        nc.tensor.matmul(
                            av[0:VSTR, avo:avo + NTOK],
                            va,
                            at[0:rows, ato + c * NTOK:ato + (c + 1) * NTOK],
                            start=(c == 0), stop=(c == 1),
                        )
                # the custom-DVE reciprocal requires a base-0 SBUF operand:
                # bounce the denominator row through dsb (ACT/DVE alternate).
                # (Pair-merging recip+broadcast across two units was tried:
                # the bursty DVE overruns step boundaries - 124us vs 110.)
                dsb = rpool.tile([1, TCH], fp32, tag="dsb", name="dsb")
                if u % 2 == 0:
                    nc.scalar.copy(dsb[:], av[64:65, 0:TCH])
                else:
                    nc.vector.tensor_copy(dsb[:], av[64:65, 0:TCH])
                rr = rpool.tile([1, TCH], fp32, tag="rr", name="rr")
                nc.vector.reciprocal_approx_fast(rr[:], dsb[:])
                avs_t[u] = av
                rr_t[u] = rr

            def norm_unit(u):
                g, p = divmod(u, 4)
                fr = FORDER[g]
                c0 = fr * NTOK
                avs = avs_t.pop(u)
                rr = rr_t.pop(u)
                rbb = rpool.tile([64, TCH], fp32, tag="rbb", name="rbb")
                # GpSimd runs ONLY partition_broadcast: mixing it with other
                # Pool ops forces a ~6us engine mode reconfig per switch.
                nc.gpsimd.partition_broadcast(rbb[:], rr[:])
                nc.vector.tensor_mul(
                    outT[p][0:64, c0:c0 + NTOK],
                    avs[0:64, 0:NTOK],
                    rbb[:, 0:NTOK],
                )
                nc.vector.tensor_mul(
                    outT[p][64:128, c0:c0 + NTOK],
                    avs[0:64, NTOK:2 * NTOK],
                    rbb[:, NTOK:2 * NTOK],
                )

            out_ps = {}

            def out_mm(mt, p, pool, tag):
                t0 = mt * 128
                msz = min(128, N - t0)
                if p == 0:
                    out_ps[mt] = pool.tile([128, DIM], fp32, tag=tag,
                                           name="chps")
                ps = out_ps[mt]
                nc.tensor.matmul(
                    ps[0:msz, :],
                    outT[p][:, t0:t0 + msz],
                    wout[p][:],
                    start=(p == 0), stop=(p == 3 and not use_bias),
                )
                if p == 3:
                    if use_bias:
                        nc.tensor.matmul(
                            ps[0:msz, :], ones_r[:, 0:msz], boutt[:],
                            start=False, stop=True,
                        )
                    ys = yspool.tile([128, DIM], fp16, tag="ys", name="ys")
                    drain()(ys[0:msz, :], ps[0:msz, :])
                    # the three final stores land back-to-back: issue the two
                    # big ones on the otherwise-idle weight queue so the
                    # issues overlap
                    q = nc.scalar if mt == 9 else nc.sync
                    q.dma_start(out=out_d[t0:t0 + msz, :], in_=ys[0:msz, :])
                    del out_ps[mt]

            def out_chain(mt):
                for p in range(4):
                    out_mm(mt, p, pmm, "mm")

            # ---- opening region: kc-outer over four live psum tiles so the
            # matmuls issue as each (wqk half, xt chunk) DMA lands; sims 0-3
            # start as soon as their chains drain ----
            first = (0, 4, 1, 5)
            ps4 = [(pmm.tile([128, DIM], fp32, tag="mm", name="p80"), 0),
                   (pmm.tile([128, DIM], fp32, tag="mm", name="p81"), 0),
                   (pav.tile([128, DIM], fp32, tag="av", name="p82"), 0),
                   (pav.tile([128, DIM], fp32, tag="av", name="p83"), 0)]
            opener_fills = {0: 2, 1: 4, 2: 3}
            for kc in range(KC):
                for c, m in enumerate(first):
                    col0 = 256 * m if m < 4 else 256 * (m - 4) + 128
                    t, b0 = ps4[c]
                    nc.tensor.matmul(
                        t[:, b0:b0 + TCH],
                        wqk[kc][:, col0:col0 + 128],
                        xt[kc][:, 0:TCH],
                        start=(kc == 0), stop=(kc == KC - 1),
                    )
                if kc in opener_fills:
                    t, b0 = ps4[kc]
                    fill(t[:, b0:b0 + TCH], opener_fills[kc], TCH)
            for c, m in enumerate(first):
                t, b0 = ps4[c]
                drain()(qkvT[m][:, 0:TCH], t[:, b0:b0 + TCH])

            sim_unit(0)
            qk_chain(2, 0)
            qk_chain(6, 0)
            sim_unit(1)
            qk_chain(3, 0)
            qk_chain(7, 0)
            v_chain(0, 0, nfill=3)
            v_chain(0, 1)
            sim_unit(2)
            av_unit(0)
            qk_chain(4, 1, nfill=2)
            qk_chain(5, 1)
            sim_unit(3)
            av_unit(1)
            norm_unit(0)
            v_chain(1, 0)
            v_chain(1, 1)

            # ---- steady-state steps s=4..34 ----
            # one q/k chain per step. Deadlines: qT pair p of token-chunk nch
            # is read by sim step 8*nch+p, but kT pair p of chunk nch is read
            # 4 steps EARLIER (step 8*nch-4+p): the jc1 stationary window of
            # frame 2*nch-1 crosses 60 columns into chunk nch. So each
            # chunk's kT chains (m=4..7) go first. v chains on even steps
            # (frame fr needed by av at step 4*fr+2), exactly one out-proj
            # chain per step at/after its earliest-ready step 4*g*+6.
            qk_sched = {}
            for i, m in enumerate((6, 7, 0, 1, 2, 3)):
                qk_sched[4 + i] = (m, 1)
            for i, m in enumerate((4, 5, 6, 7, 0, 1, 2, 3)):
                qk_sched[10 + i] = (m, 2)
                qk_sched[18 + i] = (m, 3)
            # v chain slots avoid the out-chain steps so pmm never sees three
            # allocations in one step; deadlines (emit <= av step - 1) hold.
            # Late frames are deferred toward their deadlines so the tail
            # steps keep independent PE work (the p-state clock drops on any
            # idle, so a starved tail runs everything at half clock).
            vsched = {}
            vslots = {2: (4, 5), 3: (6, 8), 4: (9, 10), 5: (12, 14),
                      6: (16, 17), 7: (20, 21)}
            for fr, (s0, s1) in vslots.items():
                vsched[s0] = (fr, 0)
                vsched[s1] = (fr, 1)
            # whole out chains at their earliest-ready step 4*g*+6 (g* = last
            # frame overlapping the chain's 128 tokens); the three chains
            # needing frame 7 (mt 10-12) are emitted INCREMENTALLY: matmul p
            # goes in right after norm(28+p) so the accumulation retires as
            # the last norms land instead of serializing after norm(31).
            out_sched = {7: 0, 11: 1, 13: 2, 15: 3, 18: 4, 19: 5, 22: 6,
                         26: 7, 27: 8}
            out_split = {11: (27, 28, 29, 30), 12: (28, 29, 30, 30),
                         9: (31, 32, 33, 34), 10: (31, 32, 33, 34)}

            for s in range(4, 35):
                if s < NU:
                    sim_unit(s)
                if s in qk_sched:
                    qk_chain(*qk_sched.pop(s))
                if s - 2 < NU:
                    av_unit(s - 2)
                if s in vsched:
                    v_chain(*vsched.pop(s))
                norm_unit(s - 3)
                if s in out_sched:
                    out_chain(out_sched.pop(s))
                for mt, psteps in out_split.items():
                    for p in range(4):
                        if psteps[p] == s:
                            out_mm(mt, p, pmm, "mm")
            assert not qk_sched and not vsched and not out_sched
            assert not out_ps

    nc.compile()
    return nc


def _get_program(use_bias: bool):
    key = ("nc", use_bias)
    if key not in _cache:
        _cache[key] = _build_bass(use_bias)
    return _cache[key]


def kernel(x=None, Wqkv=None, Wout=None, bout=None, f=None, **_unused):
    x = np.asarray(x, np.float32)
    Wqkv = np.asarray(Wqkv, np.float32)
    Wout = np.asarray(Wout, np.float32)
    bout = np.asarray(bout, np.float32)
    assert x.shape == (B, N, DIM) and int(f) == F

    wq = Wqkv.copy()
    wq[:, :DIM] *= D ** -0.5                       # fold q scaling into Wq
    # interleave q/k pair-major: [q_p(128) | k_p(128)] per pair, then v
    qk = wq[:, :2 * DIM].reshape(DIM, 2, 4, 128)   # [dim, q/k, pair, 128]
    qk = qk.transpose(0, 2, 1, 3).reshape(DIM, 2 * DIM)
    wq = np.concatenate([qk, wq[:, 2 * DIM:]], axis=1)
    wq16 = wq.astype(np.float16)
    wout16 = Wout.astype(np.float16)
    use_bias = bool(np.any(bout != 0.0))

    nc = _get_program(use_bias)

    in_maps = []
    for b in range(B):
        m = {
            "xT": np.ascontiguousarray(x[b].T).astype(np.float16),
            "wqkv": wq16,
            "wout": wout16,
        }
        if use_bias:
            m["boutr"] = bout.reshape(1, DIM).astype(np.float16)
        in_maps.append(m)

    from concourse.bass_utils import run_bass_kernel_spmd

    res = run_bass_kernel_spmd(nc, in_maps, core_ids=list(range(B)))
    return np.stack(
        [np.asarray(res.results[b]["out"], np.float32) for b in range(B)], axis=0
    )


# revision 46
# speedup vs baseline: 1.0280x; 1.0280x over previous
"""Trainium2 Bass kernel for axial (per-frame) spatial multi-head attention.

Computation (per batch element b):
    qkv = x @ Wqkv ; q,k,v heads of 64 dims, q scaled by D**-0.5
    per (head, frame): attn = softmax(q @ k^T) over 196 spatial tokens
    out = attn @ v ; y = concat-heads(out) @ Wout + bout

Sharding: pure data-parallel over batch B=8 -> one NeuronCore per batch
element, no collectives. Each core computes its full [1568, 512] output.

Single-core dataflow (no on-device transposes anywhere):
  - host supplies x^T [512,1568] fp16; q/k produced TRANSPOSED (qT/kT
    [64h, t]) with Wq/k slices stationary; v produced NATURAL with xT
    slices stationary. All PE matmuls fp16/bf16 (1 cy/row).
  - attention is FRAME-MAJOR (unit u = 4*frame + pair) and software-
    pipelined: sim(u) at step u, AV(u) at lag 2, normalize at lag 3.
  - sim per (pair, frame): four K=64 matmuls alternating PE quadrants
    (head-even rows 0:64, head-odd 64:128 -> loads hide under streaming);
    one ACT exp (bias=-SHIFT) over both heads writes attnwT bf16.
  - AV contracts j on partitions with a per-head ones column in v; row 64
    of the psum output is the softmax denominator.
  - normalize: the denominator row is bounced to a base-0 SBUF row
    (ACT/DVE alternating; the custom-DVE reciprocal faults on PSUM
    operands and silently corrupts on base!=0 operands), DVE
    reciprocal_approx_fast, GpSimd partition_broadcast (GpSimd cannot
    touch PSUM and must run only this one op kind - mixing Pool op types
    costs a ~6us engine reconfig per switch), then two lane-shifted DVE
    muls write the normalized out^T fp16 tile.
  - scheduling (the TRN2 clock governor runs k=8/full speed only after
    several us of near-continuous PE busy; any gap over ~1.5us drops it
    to k=4/half speed - visible as the ham records in the NTFF profile):
    the projection and out-proj chains are spread UNIFORMLY across the
    whole attention pipeline instead of front-loaded - one q/k chain per
    step through step 25 (each chunk's kT chains 4 steps before its qT
    chains: the jc1 stationary window of frame 2*nch-1 reads 60 columns
    into chunk nch), v chains on even steps, one out-proj chain per step
    at its earliest data-ready step. Frames run in order 0-5,7,6 (see
    FORDER): frame 6 goes last so only TWO chains (mt9, mt10) gate on
    the final norms - they emit their matmuls INCREMENTALLY (matmul p
    right after norm(28+p), steps 31-34) while the frame-7-only chains
    mt11/mt12 retire early inside the former tail-gap region.
    sims 0-3 start inside the opening region right after their q/k
    chains drain; av units 30-31 borrow the idle sim psum tiles so they
    skip the ~2.5us norm-chain wait for a pav slot.
  - startup: the opening kc-outer block needs only wqk[kc][:, 0:512]
    (pairs 0,1) + xt[kc] nch0, so the first DMAs are a 128KB wqk0 half
    and the 100KB x chunk; wqk is striped over both queues. ZERO-
    STATIONARY filler matmuls (accumulate +0 into the open psum groups)
    pad the opener's DMA-arrival stalls so the clock ramps by ~14us
    instead of ~27us - worth ~4us of total runtime.
  - DMA: weights on the ACT queue, x^T + stores on SP. GpSimd-issued
    DMA silently corrupts on this runtime; DVE/PE cannot issue DMA.
  - measured dead ends (all slower on hw): 8-wide kc-outer opener across
    all 8 psum banks (+18us!); nc.vector.reciprocal instead of the
    custom approx_fast (+60us); pair-merged recip/broadcast (+14us,
    bursty DVE); deferring v chains into the tail; norm-before-av
    emission; zero-fills inside the tail out-proj groups. The engine-
    balance here (DVE ~58%, ACT ~56%, PE ~92% of the window) is a sharp
    local optimum - single-step schedule shuffles swing +-5-15us.
"""

import numpy as np

B, N, DIM = 8, 1568, 512
H, D, F = 8, 64, 8
NTOK = 196          # spatial tokens per frame
TCH = 392           # token chunk (2 frames), 4*392=1568
KC = 4              # 128-row chunks over DIM contraction
SHIFT = 90.0        # softmax exp shift (see module docstring)
VSTR = 65           # per-head stride in v_aug (64 dims + ones column)
NU = 4 * F          # attention units (pair, frame-slot)
FORDER = (0, 1, 2, 3, 4, 5, 7, 6)  # frame processed at each slot: frame 6
# goes LAST so only two out-proj chains (mt9, mt10) gate on the final
# norms; the frame-7-only chains (mt11, mt12) retire early and their
# matmuls/drains/stores fill what used to be the tail's idle gaps

_cache = {}


def _build_bass(use_bias: bool):
    import concourse.tile as tile
    import concourse.mybir as mybir
    from concourse import bacc

    fp32 = mybir.dt.float32
    fp16 = mybir.dt.float16
    bf16 = mybir.dt.bfloat16
    Exp = mybir.ActivationFunctionType.Exp

    nc = bacc.Bacc()
    xT_d = nc.declare_dram_parameter("xT", [DIM, N], fp16, isOutput=False)
    wqkv_d = nc.declare_dram_parameter("wqkv", [DIM, 3 * DIM], fp16, isOutput=False)
    wout_d = nc.declare_dram_parameter("wout", [DIM, DIM], fp16, isOutput=False)
    if use_bias:
        bout_d = nc.declare_dram_parameter("boutr", [1, DIM], fp16, isOutput=False)
    out_d = nc.declare_dram_parameter("out", [N, DIM], fp16, isOutput=True)

    with tile.TileContext(nc) as tc:
        with (
            tc.tile_pool(name="weights", bufs=1) as wpool,
            tc.tile_pool(name="acts", bufs=1) as apool,
            tc.tile_pool(name="attnw", bufs=8) as atpool,
            tc.tile_pool(name="rows", bufs=4) as rpool,
            tc.tile_pool(name="ys", bufs=6) as yspool,
            tc.tile_pool(name="pmm", bufs=2, space="PSUM") as pmm,
            tc.tile_pool(name="psim", bufs=2, space="PSUM") as psim,
            tc.tile_pool(name="pav", bufs=2, space="PSUM") as pav,
        ):
            # ---- resident tiles ----
            # host packs wqkv columns pair-major: [q_p0|k_p0|q_p1|k_p1|...|v]
            wqk = [wpool.tile([128, 2 * DIM], fp16, tag=f"wqk_{kc}",
                              name=f"wqk_{kc}") for kc in range(KC)]
            xt = [wpool.tile([128, N], fp16, tag=f"xt_{kc}", name=f"xt_{kc}")
                  for kc in range(KC)]
            wv = [wpool.tile([128, DIM], fp16, tag=f"wv_{kc}", name=f"wv_{kc}")
                  for kc in range(KC)]
            wout = [wpool.tile([128, DIM], fp16, tag=f"wout_{p}",
                               name=f"wout_{p}") for p in range(4)]

            # ---- DMA issue plan (see module docstring) ----
            # ACT queue: wqk0 halves first (opening block), then wqk2/3, wv,
            # wout. SP queue: xt nch0 chunks interleaved with wqk1 halves,
            # then xt nch1, then merged nch2+3.
            nc.scalar.dma_start(out=wqk[0][:, 0:512], in_=wqkv_d[0:128, 0:512])
            nc.sync.dma_start(out=xt[0][:, 0:TCH], in_=xT_d[0:128, 0:TCH])
            nc.scalar.dma_start(out=wqk[0][:, 512:1024],
                                in_=wqkv_d[0:128, 512:1024])
            nc.sync.dma_start(out=wqk[1][:, 0:512], in_=wqkv_d[128:256, 0:512])
            nc.scalar.dma_start(out=wqk[2][:, 0:512],
                                in_=wqkv_d[256:384, 0:512])
            nc.sync.dma_start(out=xt[1][:, 0:TCH], in_=xT_d[128:256, 0:TCH])
            nc.scalar.dma_start(out=wqk[2][:, 512:1024],
                                in_=wqkv_d[256:384, 512:1024])
            nc.sync.dma_start(out=wqk[1][:, 512:1024],
                              in_=wqkv_d[128:256, 512:1024])
            nc.sync.dma_start(out=xt[2][:, 0:TCH], in_=xT_d[256:384, 0:TCH])
            nc.sync.dma_start(out=xt[3][:, 0:TCH], in_=xT_d[384:512, 0:TCH])
            nc.sync.dma_start(out=wqk[3][:, 0:512], in_=wqkv_d[384:512, 0:512])
            nc.sync.dma_start(out=wqk[3][:, 512:1024],
                              in_=wqkv_d[384:512, 512:1024])
            for kc in range(KC):
                nc.scalar.dma_start(
                    out=wv[kc][:], in_=wqkv_d[kc * 128:(kc + 1) * 128,
                                              2 * DIM:3 * DIM]
                )
            for kc in range(KC):
                nc.sync.dma_start(
                    out=xt[kc][:, TCH:2 * TCH],
                    in_=xT_d[kc * 128:(kc + 1) * 128, TCH:2 * TCH],
                )
            for p in range(4):
                nc.scalar.dma_start(out=wout[p][:],
                                    in_=wout_d[p * 128:(p + 1) * 128, :])
            for kc in range(KC):
                nc.sync.dma_start(
                    out=xt[kc][:, 2 * TCH:N],
                    in_=xT_d[kc * 128:(kc + 1) * 128, 2 * TCH:N],
                )
            if use_bias:
                boutt = wpool.tile([1, DIM], fp16, tag="boutr", name="boutr")
                nc.scalar.dma_start(out=boutt[:], in_=bout_d[:])
                ones_r = wpool.tile([1, 128], fp16, tag="ones_r", name="ones_r")
                nc.gpsimd.memset(ones_r[:], 1.0)
            negshift = wpool.tile([128, 1], fp32, tag="negshift", name="negshift")
            nc.gpsimd.memset(negshift[:], -SHIFT)

            # qT tiles m=0..3 (pair m heads 2m,2m+1); kT tiles m=4..7 with 64
            # zero pad columns so the jc1 stationary slice of the last frame
            # stays in bounds (rows 68:128 of jc1 sim output are garbage,
            # never read downstream).
            qkvT = [apool.tile([128, N if m < 4 else N + 64], fp16,
                               tag=f"qkvT_{m}", name=f"qkvT_{m}")
                    for m in range(8)]
            for m in range(4, 8):
                nc.gpsimd.memset(qkvT[m][:, N:N + 64], 0.0)
            vaug = []
            for fr in range(F):
                pair = []
                for c, rows in ((0, 128), (1, 68)):
                    t = apool.tile([rows, H * VSTR], bf16, tag=f"vaug_{fr}_{c}",
                                   name=f"vaug_{fr}_{c}")
                    nc.gpsimd.memset(
                        t[:].rearrange("p (h c) -> p h c", h=H)[:, :, 64:65], 1.0
                    )
                    pair.append(t)
                vaug.append(pair)
            outT = [apool.tile([128, N], fp16, tag=f"outT_{p}", name=f"outT_{p}")
                    for p in range(4)]

            drains = [nc.scalar.copy, nc.vector.tensor_copy]
            _di = [0]

            def drain():
                _di[0] += 1
                return drains[_di[0] % 2]

            def qk_chain(m, nch, nfill=0):
                col0 = 256 * m if m < 4 else 256 * (m - 4) + 128
                ps = pmm.tile([128, DIM], fp32, tag="mm", name="chps")
                if nfill:
                    fill(ps[:, 0:TCH], nfill, TCH, start=True)
                for kc in range(KC):
                    nc.tensor.matmul(
                        ps[:, 0:TCH],
                        wqk[kc][:, col0:col0 + 128],
                        xt[kc][:, nch * TCH:(nch + 1) * TCH],
                        start=(kc == 0 and not nfill), stop=(kc == KC - 1),
                    )
                drain()(qkvT[m][:, nch * TCH:(nch + 1) * TCH], ps[:, 0:TCH])

            def v_chain(fr, c, nfill=0):
                rows = 128 if c == 0 else 68
                tok0 = fr * NTOK + c * 128
                ps = pmm.tile([128, DIM], fp32, tag="mm", name="chps")
                if nfill:
                    fill(ps[:, 0:DIM], nfill, DIM, start=True)
                for kc in range(KC):
                    nc.tensor.matmul(
                        ps[0:rows, :],
                        xt[kc][:, tok0:tok0 + rows],
                        wv[kc][:],
                        start=(kc == 0 and not nfill), stop=(kc == KC - 1),
                    )
                drain()(
                    vaug[fr][c][:].rearrange("p (h c) -> p h c", h=H)[:, :, 0:64],
                    ps[0:rows, :].rearrange("p (h c) -> p h c", h=H),
                )

            # ---- attention pipeline units ----
            at_t, avs_t, rr_t, dsb_t = {}, {}, {}, {}

            # zero-stationary filler matmuls: accumulate +0 into an open
            # PSUM group while a weight/x DMA is still in flight, so the PE
            # stream never breaks during startup (any multi-us idle drops
            # the clock state and the re-ramp costs ~5us at half speed)
            zstat = wpool.tile([128, 128], fp16, tag="zstat", name="zstat")
            zmov = wpool.tile([128, DIM], fp16, tag="zmov", name="zmov")
            nc.gpsimd.memset(zstat[:], 0.0)
            nc.gpsimd.memset(zmov[:], 0.0)

            def fill(ps_ap, n, w, start=False):
                for i in range(n):
                    nc.tensor.matmul(
                        ps_ap, zstat[:], zmov[:, 0:w],
                        start=(start and i == 0), stop=False,
                    )

            def sim_unit(u):
                g, p = divmod(u, 4)
                fr = FORDER[g]
                c0 = fr * NTOK
                # head blocks at 512-col stride: each head's 392 sim columns
                # stay inside one 2KB PSUM bank (matmul dst cannot span banks)
                ps = psim.tile([128, 1024], fp32, tag="sim", name="sim")
                qTt, kTt = qkvT[p], qkvT[4 + p]
                for hh, jc in ((0, 0), (1, 0), (0, 1), (1, 1)):
                    base = hh * 64
                    off = hh * 512 + jc * NTOK
                    nc.tensor.matmul(
                        ps[0:128, off:off + NTOK],
                        kTt[base:base + 64, c0 + jc * 128:c0 + jc * 128 + 128],
                        qTt[base:base + 64, c0:c0 + NTOK],
                    )
                at = atpool.tile([128, 2 * TCH], bf16, tag="at", name="at")
                nc.scalar.activation(
                    at[:].rearrange("p (b c) -> p b c", b=2),
                    ps[:].rearrange("p (b c) -> p b c", b=2)[:, :, 0:TCH],
                    Exp,
                    bias=negshift[:],
                )
                at_t[u] = at

            def av_unit(u):
                g, p = divmod(u, 4)
                fr = FORDER[g]
                at = at_t.pop(u)
                if u >= 30:
                    # sims have stopped allocating by now: borrow the sim
                    # psum tiles so the last av units don't wait on the
                    # ~2.5us norm chain to free a pav slot (the tail's PE
                    # gaps drop the clock to half speed)
                    av = psim.tile([128, 1024], fp32, tag="sim", name="av")
                else:
                    av = pav.tile([128, DIM], fp32, tag="av", name="av")
                for hh in range(2):
                    ato = hh * TCH
                    avo = hh * NTOK
                    for c, rows in ((0, 128), (1, 68)):
                        va = vaug[fr][c][:].rearrange(
                            "p (h c) -> p h c", h=H)[:, 2 * p + hh, :]
                        nc.tensor.matmul(
                            av[0:VSTR, avo:avo + NTOK],
                            va,
                            at[0:rows, ato + c * NTOK:ato + (c + 1) * NTOK],
                            start=(c == 0), stop=(c == 1),
                        )
                # the custom-DVE reciprocal requires a base-0 SBUF operand:
                # bounce the denominator row through dsb (ACT/DVE alternate).
                # (Pair-merging recip+broadcast across two units was tried:
                # the bursty DVE overruns step boundaries - 124us vs 110.)
                dsb = rpool.tile([1, TCH], fp32, tag="dsb", name="dsb")
                if u % 2 == 0:
                    nc.scalar.copy(dsb[:], av[64:65, 0:TCH])
                else:
                    nc.vector.tensor_copy(dsb[:], av[64:65, 0:TCH])
                rr = rpool.tile([1, TCH], fp32, tag="rr", name="rr")
                nc.vector.reciprocal_approx_fast(rr[:], dsb[:])
                avs_t[u] = av
                rr_t[u] = rr

            def norm_unit(u):
                g, p = divmod(u, 4)
                fr = FORDER[g]
                c0 = fr * NTOK
                avs = avs_t.pop(u)
                rr = rr_t.pop(u)
                rbb = rpool.tile([64, TCH], fp32, tag="rbb", name="rbb")
                # GpSimd runs ONLY partition_broadcast: mixing it with other
                # Pool ops forces a ~6us engine mode reconfig per switch.
                nc.gpsimd.partition_broadcast(rbb[:], rr[:])
                nc.vector.tensor_mul(
                    outT[p][0:64, c0:c0 + NTOK],
                    avs[0:64, 0:NTOK],
                    rbb[:, 0:NTOK],
                )
                nc.vector.tensor_mul(
                    outT[p][64:128, c0:c0 + NTOK],
                    avs[0:64, NTOK:2 * NTOK],
                    rbb[:, NTOK:2 * NTOK],
                )

            out_ps = {}

            def out_mm(mt, p, pool, tag):
                t0 = mt * 128
                msz = min(128, N - t0)
                if p == 0:
                    out_ps[mt] = pool.tile([128, DIM], fp32, tag=tag,
                                           name="chps")
                ps = out_ps[mt]
                nc.tensor.matmul(
                    ps[0:msz, :],
                    outT[p][:, t0:t0 + msz],
                    wout[p][:],
                    start=(p == 0), stop=(p == 3 and not use_bias),
                )
                if p == 3:
                    if use_bias:
                        nc.tensor.matmul(
                            ps[0:msz, :], ones_r[:, 0:msz], boutt[:],
                            start=False, stop=True,
                        )
                    ys = yspool.tile([128, DIM], fp16, tag="ys", name="ys")
                    drain()(ys[0:msz, :], ps[0:msz, :])
                    # the three final stores land back-to-back: issue the two
                    # big ones on the otherwise-idle weight queue so the
                    # issues overlap
                    q = nc.scalar if mt == 9 else nc.sync
                    q.dma_start(out=out_d[t0:t0 + msz, :], in_=ys[0:msz, :])
                    del out_ps[mt]

            def out_chain(mt):
                for p in range(4):
                    out_mm(mt, p, pmm, "mm")

            # ---- opening region: kc-outer over four live psum tiles so the
            # matmuls issue as each (wqk half, xt chunk) DMA lands; sims 0-3
            # start as soon as their chains drain ----
            first = (0, 4, 1, 5)
            ps4 = [(pmm.tile([128, DIM], fp32, tag="mm", name="p80"), 0),
                   (pmm.tile([128, DIM], fp32, tag="mm", name="p81"), 0),
                   (pav.tile([128, DIM], fp32, tag="av", name="p82"), 0),
                   (pav.tile([128, DIM], fp32, tag="av", name="p83"), 0)]
            opener_fills = {0: 2, 1: 4, 2: 3}
            # PE warmup: these fills have NO DMA dependency (zstat/zmov are
            # memsets), so they start right after the preamble (~7.5us)
            # while the first weight DMA is still in flight - the clock
            # governor is already ramping when real work begins at ~9.8us
            tw, bw = ps4[0]
            fill(tw[:, bw:bw + TCH], 8, TCH, start=True)
            for kc in range(KC):
                for c, m in enumerate(first):
                    col0 = 256 * m if m < 4 else 256 * (m - 4) + 128
                    t, b0 = ps4[c]
                    nc.tensor.matmul(
                        t[:, b0:b0 + TCH],
                        wqk[kc][:, col0:col0 + 128],
                        xt[kc][:, 0:TCH],
                        start=(kc == 0 and c != 0), stop=(kc == KC - 1),
                    )
                if kc in opener_fills:
                    t, b0 = ps4[kc]
                    fill(t[:, b0:b0 + TCH], opener_fills[kc], TCH)
            for c, m in enumerate(first):
                t, b0 = ps4[c]
                drain()(qkvT[m][:, 0:TCH], t[:, b0:b0 + TCH])

            sim_unit(0)
            qk_chain(2, 0)
            qk_chain(6, 0)
            sim_unit(1)
            qk_chain(3, 0)
            qk_chain(7, 0)
            v_chain(0, 0, nfill=3)
            v_chain(0, 1)
            sim_unit(2)
            av_unit(0)
            qk_chain(4, 1, nfill=2)
            qk_chain(5, 1)
            sim_unit(3)
            av_unit(1)
            norm_unit(0)
            v_chain(1, 0)
            v_chain(1, 1)

            # ---- steady-state steps s=4..34 ----
            # one q/k chain per step. Deadlines: qT pair p of token-chunk nch
            # is read by sim step 8*nch+p, but kT pair p of chunk nch is read
            # 4 steps EARLIER (step 8*nch-4+p): the jc1 stationary window of
            # frame 2*nch-1 crosses 60 columns into chunk nch. So each
            # chunk's kT chains (m=4..7) go first. v chains on even steps
            # (frame fr needed by av at step 4*fr+2), exactly one out-proj
            # chain per step at/after its earliest-ready step 4*g*+6.
            qk_sched = {}
            for i, m in enumerate((6, 7, 0, 1, 2, 3)):
                qk_sched[4 + i] = (m, 1)
            for i, m in enumerate((4, 5, 6, 7, 0, 1, 2, 3)):
                qk_sched[10 + i] = (m, 2)
                qk_sched[18 + i] = (m, 3)
            # v chain slots avoid the out-chain steps so pmm never sees three
            # allocations in one step; deadlines (emit <= av step - 1) hold.
            # Late frames are deferred toward their deadlines so the tail
            # steps keep independent PE work (the p-state clock drops on any
            # idle, so a starved tail runs everything at half clock).
            vsched = {}
            vslots = {2: (4, 5), 3: (6, 8), 4: (9, 10), 5: (12, 14),
                      6: (16, 17), 7: (20, 21)}
            for fr, (s0, s1) in vslots.items():
                vsched[s0] = (fr, 0)
                vsched[s1] = (fr, 1)
            # whole out chains at their earliest-ready step 4*g*+6 (g* = last
            # frame overlapping the chain's 128 tokens); the three chains
            # needing frame 7 (mt 10-12) are emitted INCREMENTALLY: matmul p
            # goes in right after norm(28+p) so the accumulation retires as
            # the last norms land instead of serializing after norm(31).
            out_sched = {7: 0, 11: 1, 13: 2, 15: 3, 18: 4, 19: 5, 22: 6,
                         26: 7, 27: 8}
            out_split = {11: (27, 28, 29, 30), 12: (28, 29, 30, 30),
                         9: (31, 32, 33, 34), 10: (31, 32, 33, 34)}

            for s in range(4, 35):
                if s < NU:
                    sim_unit(s)
                if s in qk_sched:
                    qk_chain(*qk_sched.pop(s))
                if s - 2 < NU:
                    av_unit(s - 2)
                if s in vsched:
                    v_chain(*vsched.pop(s))
                norm_unit(s - 3)
                if s in out_sched:
                    out_chain(out_sched.pop(s))
                for mt, psteps in out_split.items():
                    for p in range(4):
                        if psteps[p] == s:
                            out_mm(mt, p, pmm, "mm")
            assert not qk_sched and not vsched and not out_sched
            assert not out_ps

    nc.compile()
    return nc


def _get_program(use_bias: bool):
    key = ("nc", use_bias)
    if key not in _cache:
        _cache[key] = _build_bass(use_bias)
    return _cache[key]


def kernel(x=None, Wqkv=None, Wout=None, bout=None, f=None, **_unused):
    x = np.asarray(x, np.float32)
    Wqkv = np.asarray(Wqkv, np.float32)
    Wout = np.asarray(Wout, np.float32)
    bout = np.asarray(bout, np.float32)
    assert x.shape == (B, N, DIM) and int(f) == F

    wq = Wqkv.copy()
    wq[:, :DIM] *= D ** -0.5                       # fold q scaling into Wq
    # interleave q/k pair-major: [q_p(128) | k_p(128)] per pair, then v
    qk = wq[:, :2 * DIM].reshape(DIM, 2, 4, 128)   # [dim, q/k, pair, 128]
    qk = qk.transpose(0, 2, 1, 3).reshape(DIM, 2 * DIM)
    wq = np.concatenate([qk, wq[:, 2 * DIM:]], axis=1)
    wq16 = wq.astype(np.float16)
    wout16 = Wout.astype(np.float16)
    use_bias = bool(np.any(bout != 0.0))

    nc = _get_program(use_bias)

    in_maps = []
    for b in range(B):
        m = {
            "xT": np.ascontiguousarray(x[b].T).astype(np.float16),
            "wqkv": wq16,
            "wout": wout16,
        }
        if use_bias:
            m["boutr"] = bout.reshape(1, DIM).astype(np.float16)
        in_maps.append(m)

    from concourse.bass_utils import run_bass_kernel_spmd

    res = run_bass_kernel_spmd(nc, in_maps, core_ids=list(range(B)))
    return np.stack(
        [np.asarray(res.results[b]["out"], np.float32) for b in range(B)], axis=0
    )
